# revision 2
# baseline (speedup 1.0000x reference)
"""Trainium2 Bass kernel for nn_GCNStacking: 3-layer dense-adjacency GraphConv.

Per batch element b (one per NeuronCore, B=8 = n_cores=8, pure data parallel):
    H = relu(A @ (X @ Wm0^T) + X @ Ws0^T + b0)
    H = relu(A @ (H @ Wm1^T) + H @ Ws1^T + b1)
    H =      A @ (H @ Wm2^T) + H @ Ws2^T + b2

v2 design (all-bf16 residency):
  - A is cast f32->bf16 DURING the DMA load (SWDGE) in 16 x 1MB-source
    slabs; measured at ~413 GB/s aggregate, same as plain HWDGE loads.
  - A^T materialized once in SBUF (bf16, 8MB) via plain-matmul transposes
    (lhsT = A-block, rhs = bf16 identity): bf16 LDWEIGHTS gets FWL, so a
    128x128 block costs ~80ns vs ~136ns for transpose-mode, at the price
    of f32 PSUM evacuation casts.
  - Aggregation: Ot[c, i-chunk] accumulated over 16 j-blocks, lhsT =
    M-block [128,64] bf16, rhs = A^T chunk [128,512] bf16: 110ns/MM
    measured (2 cols/cycle, LDW hidden). No column packing needed.
  - Evacuations: bias+relu on ACT straight into transposed state Ht[l+1];
    transpose evac casts split DVE:ACT to balance engines.
  - Final layer transposed back to natural [N, C] f32 via PE, DMA'd out
    per 512-row chunk.
"""
import sys

for _p in ("/opt/trn_rl_repo",):
    if _p not in sys.path:
        sys.path.insert(0, _p)

import numpy as np
import orjson

import concourse.bass as bass
import concourse.tile as tile
from concourse import mybir
from concourse.bass import _add_dep_helper as add_dep

f32 = mybir.dt.float32
bf16 = mybir.dt.bfloat16

# ---------------------------------------------------------------------------
# Workaround: this walrus build accepts at most ONE embedded sync-wait per
# instruction ("Too many sync wait commands").  Split excess waits onto
# inserted NoOps (same engine, right before the host instruction).
# ---------------------------------------------------------------------------
_ws_ctr = [0]


def _split_waits_json(bir_bytes: bytes) -> bytes:
    d = orjson.loads(bir_bytes)
    changed = False
    for fn in d.get("functions", []):
        for blk in fn.get("blocks", []):
            out = []
            for inst in blk.get("instructions", []):
                si = inst.get("sync_info")
                waits = (si or {}).get("on_wait") or []
                eng = inst.get("engine")
                if len(waits) > 1 and eng and eng != "Unassigned":
                    changed = True
                    for w in waits[:-1]:
                        _ws_ctr[0] += 1
                        out.append({
                            "name": f"I-wsplit-{_ws_ctr[0]}",
                            "opcode": "NoOp",
                            "engine": eng,
                            "ins": [],
                            "outs": [],
                            "sync_info": {"on_wait": [w], "on_update": []},
                        })
                    si["on_wait"] = waits[-1:]
                out.append(inst)
            blk["instructions"] = out
    return orjson.dumps(d) if changed else bir_bytes


def _install_waitsplit():
    from concourse import bass2jax, bass_utils
    if getattr(bass_utils, "_waitsplit_installed", False):
        return
    orig = bass_utils.compile_bir_kernel

    def patched(bir_json, tmpdir, neff_name="file.neff"):
        return orig(_split_waits_json(bytes(bir_json)), tmpdir, neff_name=neff_name)

    bass_utils.compile_bir_kernel = patched
    bass2jax.compile_bir_kernel = patched
    bass_utils._waitsplit_installed = True


_install_waitsplit()

# ---------------------------------------------------------------------------
# Kernel builder
# ---------------------------------------------------------------------------
P = 128
C = 64
N_LAYERS = 3


def build_gcn(nn_nodes: int = 2048):
    """Single-core Bass program; runs SPMD on all 8 cores with per-core
    (per-batch) inputs."""
    NN = nn_nodes
    NB = NN // P            # node blocks (16)
    CH = 512                # aggregation i-chunk (one PSUM bank of f32)
    IC = NN // CH           # i-chunks (4)
    GS = CH // P            # slabs per i-chunk group (4)

    nc = bass.Bass()
    X_in = nc.declare_dram_parameter("X", [NN, C], f32, isOutput=False)
    A_in = nc.declare_dram_parameter("A", [NN, NN], f32, isOutput=False)
    W_in = {}
    b_in = {}
    for l in range(N_LAYERS):
        W_in[(l, "m")] = nc.declare_dram_parameter(f"Wm{l}", [C, C], f32, isOutput=False)
        W_in[(l, "s")] = nc.declare_dram_parameter(f"Ws{l}", [C, C], f32, isOutput=False)
        b_in[l] = nc.declare_dram_parameter(f"b{l}", [C], f32, isOutput=False)
    H_out = nc.declare_dram_parameter("H", [NN, C], f32, isOutput=True)

    with tile.TileContext(nc) as tc:
        with (
            tc.tile_pool(name="const", bufs=1) as const,
            tc.tile_pool(name="ht_pool", bufs=2) as ht_pool,
            tc.tile_pool(name="mn_pool", bufs=2) as mn_pool,
            tc.tile_pool(name="slab_pool", bufs=10) as slab_pool,
            tc.tile_pool(name="u_pool", bufs=2) as u_pool,
            tc.tile_pool(name="hb_pool", bufs=4) as hb_pool,
            tc.tile_pool(name="ps_tr", bufs=4, space="PSUM") as ps_tr,
            tc.tile_pool(name="ps_o", bufs=2, space="PSUM") as ps_o,
            tc.tile_pool(name="ps_m", bufs=2, space="PSUM") as ps_m,
        ):
            # ---- phase 0: constants, X, W loads ---------------------------
            ident = const.tile([P, P], f32, name="ident")
            id_i1 = nc.gpsimd.memset(ident, 0.0)
            id_i2 = nc.gpsimd.affine_select(
                out=ident, in_=ident,
                compare_op=mybir.AluOpType.not_equal,
                fill=1.0, base=0, pattern=[[-1, P]], channel_multiplier=1,
            )
            identb = const.tile([P, P], bf16, name="identb")
            idb = nc.vector.tensor_copy(identb, ident)

            x_sb = const.tile([P, NB, C], f32, name="x_sb")
            x_dma = nc.sync.dma_start(
                x_sb, X_in[:].rearrange("(nb p) c -> p nb c", p=P))

            w_stage = {}
            w_dmas = []
            for l in range(N_LAYERS):
                for kind in ("m", "s"):
                    wst = const.tile([C, C], f32, name=f"wst_{l}{kind}")
                    w_dmas.append(nc.sync.dma_start(wst, W_in[(l, kind)][:]))
                    w_stage[(l, kind)] = wst
            b_sb = {}
            b_dmas = []
            for l in range(N_LAYERS):
                bt = const.tile([C, 1], f32, name=f"b_sb{l}")
                b_dmas.append(nc.sync.dma_start(
                    bt, b_in[l][:].rearrange("(p o) -> p o", o=1)))
                b_sb[l] = bt

            # ---- A slab loads: cast f32->bf16 during DMA (SWDGE) ----------
            a_sl = []
            a_dmas = []
            for s in range(NB):
                t = slab_pool.tile([P, NN], bf16, name=f"asl{s}", tag="aslab")
                d = nc.gpsimd.dma_start(t, A_in[s * P:(s + 1) * P, :])
                if s == 2:
                    # slabs 0-1's descriptor gens go first (A load start is
                    # the critical path); ident setup slots in before the
                    # rest of the gen flood
                    add_dep(d.ins, id_i2.ins, False, "ident before A gen")
                if s == 3:
                    # slabs 0-2 start immediately; slab 3+ waits until the
                    # X DMA has drained so it is not starved behind 16MB of
                    # A traffic (SWDGE is FIFO; x normally completes during
                    # slabs 0-2 so this rarely pauses the queue)
                    add_dep(d.ins, x_dma.ins, True, "x DMA before A")
                a_sl.append(t)
                a_dmas.append(d)

            # granular gates: xT needs x+ident only; w gates separate
            gate_x = nc.tensor.nop(nofuse=True)
            for d in (id_i1, id_i2, idb, x_dma):
                add_dep(gate_x.ins, d.ins, True, "x gate")
            gate_w = nc.tensor.nop(nofuse=True)
            for d in w_dmas:
                add_dep(gate_w.ins, d.ins, True, "w gate")

            # warm-up matmuls: engage the PE HAM clock-gate before real work
            warm_gate = nc.tensor.nop(nofuse=True)
            for d in (id_i1, id_i2):
                add_dep(warm_gate.ins, d.ins, True, "warmup gate")
            pwarm = ps_m.tile([P, C], f32, name="pwarm", tag="m")
            for wi in range(28):
                wmm = nc.tensor.matmul(pwarm[:C, :], ident[:, :C],
                                       ident[:, :C], start=True, stop=True,
                                       skip_group_check=True)
                if wi == 0:
                    add_dep(wmm.ins, warm_gate.ins, False, "after warmup gate")

            # ---- X^T -> Ht[0] (bf16), W^T (bf16) --------------------------
            # Ht[l]: transposed state, padded to 128 partitions with zeros
            # so the self-term matmul can run k=128 as the full-bank
            # accumulation opener (see emit_agg).
            Ht = [ht_pool.tile([P, NN], bf16, name=f"Ht{l}", tag="ht")
                  for l in range(N_LAYERS)]
            # bufs=2 ring: zeroing the bottom half of the first two tiles
            # covers both physical buffers; nothing ever writes it again.
            # On DVE: gpsimd must stay free for the A-slab descriptor gen.
            nc.vector.memset(Ht[0][C:, :], 0.0)
            nc.vector.memset(Ht[1][C:, :], 0.0)

            def emit_xT():
                for q in range(NB // 4):
                    pt = ps_tr.tile([P, 4 * P], f32, name="pt_x", tag="tr")
                    for si in range(4):
                        nb = q * 4 + si
                        t = nc.tensor.matmul(pt[:C, si * P:(si + 1) * P],
                                             x_sb[:, nb, :], ident,
                                             start=True, stop=True,
                                             skip_group_check=True)
                        if q == 0 and si == 0:
                            add_dep(t.ins, gate_x.ins, False, "after gate_x")
                    nc.vector.tensor_copy(
                        Ht[0][:C, q * 4 * P:(q + 1) * 4 * P],
                        pt[:C, :].rearrange("p (si i) -> p si i", si=4))

            wT = {}
            # self-term weights live in a [128, 128] zero-padded tile so the
            # self matmul runs k=128 x m=128 (writes the whole PSUM bank)
            wsp = {}
            for l in range(N_LAYERS):
                t = const.tile([P, C], bf16, name=f"wsp{l}")
                nc.vector.memset(t, 0.0)
                wsp[l] = t

            def emit_wT():
                first_w = True
                for (l, kind), wst in w_stage.items():
                    pw = ps_tr.tile([P, 4 * P], f32, name="pt_w", tag="tr")
                    t = nc.tensor.matmul(pw[:C, :C], wst, ident[:C, :C],
                                         start=True, stop=True,
                                         skip_group_check=True)
                    if first_w:
                        add_dep(t.ins, gate_w.ins, False, "after gate_w")
                        first_w = False
                    if kind == "s":
                        nc.vector.tensor_copy(wsp[l][:C, :C], pw[:C, :C])
                    else:
                        wt = const.tile([C, C], bf16, name=f"wT_{l}{kind}")
                        nc.vector.tensor_copy(wt, pw[:C, :C])
                        wT[(l, kind)] = wt

            # resident A^T [j-partition, j-block, i], bf16
            ATr = const.tile([P, NB, NN], bf16, name="ATr")

            mn_ctr = [0]

            def emit_mprod(l, mn, jbs=None, after=None):
                """M_l natural [N, C] blocks: lhsT = Ht[l] block, rhs = WmT.

                Emitted in groups of 4 into one PSUM bank with a single
                evacuation copy, so the copies never stall the matmul
                stream.  `after`: program-order pin so the scheduler cannot
                scatter these between a chunk's agg matmuls."""
                jbs = list(range(NB) if jbs is None else jbs)
                for q0 in range(0, len(jbs), 4):
                    grp = jbs[q0:q0 + 4]
                    pm = ps_m.tile([P, 4, C], f32, name="pm", tag="m")
                    for k, jb in enumerate(grp):
                        mm = nc.tensor.matmul(
                            pm[:, k, :], Ht[l][:C, jb * P:(jb + 1) * P],
                            wT[(l, "m")], start=True, stop=True,
                            skip_group_check=True)
                        if after is not None:
                            add_dep(mm.ins, after.ins, False, "pin after agg")
                            after = None
                    dst = mn[:, grp[0]:grp[0] + len(grp), :]
                    # alternate Mn copies DVE/ACT
                    if mn_ctr[0] % 2 == 0:
                        nc.scalar.copy(dst, pm[:, :len(grp), :])
                    else:
                        nc.vector.tensor_copy(dst, pm[:, :len(grp), :])
                    mn_ctr[0] += 1

            # bridge dummies: keep PE warm across the wait for slab 0
            pbr = ps_m.tile([P, C], f32, name="pbr", tag="m")
            for _bi in range(6):
                nc.tensor.matmul(pbr[:C, :], ident[:, :C], ident[:, :C],
                                 start=True, stop=True, skip_group_check=True)

            # ---- A transposes: plain-matmul vs bf16 identity --------------
            evac_ctr = [0]

            def emit_slab_transposes(s):
                """16 block transposes of slab s into ATr[:, :, s*128:...]."""
                for q in range(NB // 4):
                    pt = ps_tr.tile([P, 4 * P], f32, name="pt_a", tag="tr")
                    for si in range(4):
                        jb = q * 4 + si
                        nc.tensor.matmul(
                            pt[:, si * P:(si + 1) * P],
                            a_sl[s][:, jb * P:(jb + 1) * P],
                            identb, start=True, stop=True,
                            skip_group_check=True)
                    dst = ATr[:, q * 4:(q + 1) * 4, s * P:(s + 1) * P]
                    src = pt[:, :].rearrange("p (si i) -> p si i", si=4)
                    # 2:1 DVE:ACT split on the cast evacuations
                    if evac_ctr[0] % 3 != 2:
                        nc.vector.tensor_copy(dst, src)
                    else:
                        nc.scalar.copy(dst, src)
                    evac_ctr[0] += 1

            ho_tiles = {}

            def emit_evac(l, g, po):
                # col-packed halves: out = po[0:64] + po[64:128] + b.
                # Engines read at most one non-scalar PSUM input per op.
                cs = slice(g * CH, (g + 1) * CH)
                v = u_pool.tile([C, CH], f32, name="v", tag="v")
                nc.scalar.activation(v, po[C:2 * C, :],
                                     mybir.ActivationFunctionType.Identity,
                                     bias=b_sb[l], scale=1.0)
                if l < N_LAYERS - 1:
                    u = u_pool.tile([C, CH], f32, name="u", tag="u")
                    nc.vector.tensor_tensor(u, po[:C, :], v,
                                            mybir.AluOpType.add)
                    nc.vector.tensor_scalar(
                        Ht[l + 1][:C, cs], u, 0.0, None, mybir.AluOpType.max)
                    return
                # final layer: bias only (no relu); transpose-out deferred
                # one chunk so the PE is not stalled on this evac chain
                ho = u_pool.tile([C, CH], f32, name="ho", tag="ho")
                nc.vector.tensor_tensor(ho, po[:C, :], v,
                                        mybir.AluOpType.add)
                ho_tiles[g] = ho

            def emit_final_out(g, after=None):
                ho = ho_tiles.pop(g)
                hb = hb_pool.tile([P, CH // P, C], f32, name="hb", tag="hb")
                for k in range(CH // P):
                    ph = ps_tr.tile([P, 4 * P], f32, name="ph", tag="tr")
                    mm = nc.tensor.matmul(ph[:, :C], ho[:, k * P:(k + 1) * P],
                                          ident[:C, :C], start=True, stop=True,
                                          skip_group_check=True)
                    if after is not None:
                        add_dep(mm.ins, after.ins, False, "pin after agg")
                        after = None
                    nc.vector.tensor_copy(hb[:, k, :], ph[:, :C])
                r0 = g * CH
                nc.sync.dma_start(
                    H_out[r0:r0 + CH, :].rearrange("(k p) c -> p k c", p=P),
                    hb)

            def emit_agg(l, g, mn):
                # col-packed: even j-blocks -> psum partitions 0:64, odd ->
                # 64:128; concurrent in the array's column groups, which
                # also lets LDWEIGHTS overlap the other half's matmul.
                cs = slice(g * CH, (g + 1) * CH)
                po = ps_o.tile([P, CH], f32, name="po", tag="o")
                # The k=128 m=64 self-term (zero-padded weights rows 64:128)
                # opens the even col-group chain; jb1 opens the odd chain
                # concurrently in the other col group.  j-blocks then
                # alternate col groups so each LDWEIGHTS hides under the
                # other group's streaming.
                last = nc.tensor.matmul(po[:C, :], wsp[l], Ht[l][:, cs],
                                        start=True, stop=False,
                                        skip_group_check=True)
                seq = []
                for k in range(NB // 2):
                    seq.append((2 * k + 1, k == 0, 2 * k + 1 == NB - 1))
                    seq.append((2 * k, False, 2 * k == NB - 2))
                for jb, st, sp in seq:
                    h = jb % 2
                    last = nc.tensor.matmul(
                        po[h * C:(h + 1) * C, :],
                        mn[:, jb, :], ATr[:, jb, cs],
                        start=st, stop=sp, skip_group_check=True)
                emit_evac(l, g, po)
                return last

            # ---- layer 1, pipelined with the A load/transpose -------------
            mns = [mn_pool.tile([P, NB, C], bf16, name=f"mn{l}", tag="mn")
                   for l in range(N_LAYERS)]

            # Mprods for layer l+1 chunk g are emitted one chunk LATE (after
            # agg of chunk g+1) so the PE never stalls on chunk g's
            # ACT/DVE evacuation chain.  wT/mprod0 are emitted after slab
            # 0's transposes: slab 0 usually lands before the W inputs.
            GB = NB // IC
            for g in range(IC):
                for s in range(g * GS, (g + 1) * GS):
                    emit_slab_transposes(s)
                    if s == 0:
                        emit_xT()
                    if s == 1:
                        emit_wT()
                        emit_mprod(0, mns[0])
                    if g > 0 and s % GS == 0:
                        la = emit_agg(0, g - 1, mns[0])
                        if g > 1:
                            jb0 = (g - 2) * GB
                            emit_mprod(1, mns[1], range(jb0, jb0 + GB),
                                       after=la)
            la = emit_agg(0, IC - 1, mns[0])
            emit_mprod(1, mns[1], range((IC - 2) * GB, (IC - 1) * GB),
                       after=la)
            emit_mprod(1, mns[1], range((IC - 1) * GB, NB))

            # ---- layer 2 --------------------------------------------------
            for g in range(IC):
                la = emit_agg(1, g, mns[1])
                if g > 0:
                    jb0 = (g - 1) * GB
                    emit_mprod(2, mns[2], range(jb0, jb0 + GB), after=la)
            emit_mprod(2, mns[2], range((IC - 1) * GB, NB))

            # ---- layer 3 + transposed output ------------------------------
            for g in range(IC):
                la = emit_agg(2, g, mns[2])
                if g > 0:
                    emit_final_out(g - 1, after=la)
            emit_final_out(IC - 1)

    return nc


# ---------------------------------------------------------------------------
# Harness entry point
# ---------------------------------------------------------------------------
_NC_CACHE = {}


def _get_nc(nn_nodes):
    if nn_nodes not in _NC_CACHE:
        _NC_CACHE[nn_nodes] = build_gcn(nn_nodes)
    return _NC_CACHE[nn_nodes]


def kernel(X, A, Wm0, Ws0, b0, Wm1, Ws1, b1, Wm2, Ws2, b2, _trace=False):
    from concourse.bass_utils import run_bass_kernel_spmd

    X = np.ascontiguousarray(np.asarray(X, dtype=np.float32))
    A = np.ascontiguousarray(np.asarray(A, dtype=np.float32))
    B, NN, _C = X.shape
    assert B == 8, f"expected batch 8 (one per core), got {B}"

    shared = {
        "Wm0": np.ascontiguousarray(np.asarray(Wm0, np.float32)),
        "Ws0": np.ascontiguousarray(np.asarray(Ws0, np.float32)),
        "b0": np.ascontiguousarray(np.asarray(b0, np.float32)),
        "Wm1": np.ascontiguousarray(np.asarray(Wm1, np.float32)),
        "Ws1": np.ascontiguousarray(np.asarray(Ws1, np.float32)),
        "b1": np.ascontiguousarray(np.asarray(b1, np.float32)),
        "Wm2": np.ascontiguousarray(np.asarray(Wm2, np.float32)),
        "Ws2": np.ascontiguousarray(np.asarray(Ws2, np.float32)),
        "b2": np.ascontiguousarray(np.asarray(b2, np.float32)),
    }
    nc = _get_nc(NN)
    in_maps = [dict(shared, X=X[b], A=A[b]) for b in range(B)]
    res = run_bass_kernel_spmd(nc, in_maps, core_ids=list(range(B)),
                               trace=_trace)
    out = np.stack([res.results[b]["H"] for b in range(B)], axis=0)
    if _trace:
        return out, res
    return out


# revision 3
# speedup vs baseline: 1.0582x; 1.0582x over previous
"""Trainium2 Bass kernel for nn_GCNStacking: 3-layer dense-adjacency GraphConv.

Per batch element b (one per NeuronCore, B=8 = n_cores=8, pure data parallel):
    H = relu(A @ (X @ Wm0^T) + X @ Ws0^T + b0)
    H = relu(A @ (H @ Wm1^T) + H @ Ws1^T + b1)
    H =      A @ (H @ Wm2^T) + H @ Ws2^T + b2

v2 design (all-bf16 residency):
  - A is cast f32->bf16 DURING the DMA load (SWDGE) in 16 x 1MB-source
    slabs; measured at ~413 GB/s aggregate, same as plain HWDGE loads.
  - A^T materialized once in SBUF (bf16, 8MB) via plain-matmul transposes
    (lhsT = A-block, rhs = bf16 identity): bf16 LDWEIGHTS gets FWL, so a
    128x128 block costs ~80ns vs ~136ns for transpose-mode, at the price
    of f32 PSUM evacuation casts.
  - Aggregation: Ot[c, i-chunk] accumulated over 16 j-blocks, lhsT =
    M-block [128,64] bf16, rhs = A^T chunk [128,512] bf16: 110ns/MM
    measured (2 cols/cycle, LDW hidden). No column packing needed.
  - Evacuations: bias+relu on ACT straight into transposed state Ht[l+1];
    transpose evac casts split DVE:ACT to balance engines.
  - Final layer transposed back to natural [N, C] f32 via PE, DMA'd out
    per 512-row chunk.
"""
import sys

for _p in ("/opt/trn_rl_repo",):
    if _p not in sys.path:
        sys.path.insert(0, _p)

import numpy as np
import orjson

import concourse.bass as bass
import concourse.tile as tile
from concourse import mybir
from concourse.bass import _add_dep_helper as add_dep

f32 = mybir.dt.float32
bf16 = mybir.dt.bfloat16

# ---------------------------------------------------------------------------
# Workaround: this walrus build accepts at most ONE embedded sync-wait per
# instruction ("Too many sync wait commands").  Split excess waits onto
# inserted NoOps (same engine, right before the host instruction).
# ---------------------------------------------------------------------------
_ws_ctr = [0]


def _split_waits_json(bir_bytes: bytes) -> bytes:
    d = orjson.loads(bir_bytes)
    changed = False
    for fn in d.get("functions", []):
        for blk in fn.get("blocks", []):
            out = []
            for inst in blk.get("instructions", []):
                si = inst.get("sync_info")
                waits = (si or {}).get("on_wait") or []
                eng = inst.get("engine")
                if len(waits) > 1 and eng and eng != "Unassigned":
                    changed = True
                    for w in waits[:-1]:
                        _ws_ctr[0] += 1
                        out.append({
                            "name": f"I-wsplit-{_ws_ctr[0]}",
                            "opcode": "NoOp",
                            "engine": eng,
                            "ins": [],
                            "outs": [],
                            "sync_info": {"on_wait": [w], "on_update": []},
                        })
                    si["on_wait"] = waits[-1:]
                out.append(inst)
            blk["instructions"] = out
    return orjson.dumps(d) if changed else bir_bytes


def _install_waitsplit():
    from concourse import bass2jax, bass_utils
    if getattr(bass_utils, "_waitsplit_installed", False):
        return
    orig = bass_utils.compile_bir_kernel

    def patched(bir_json, tmpdir, neff_name="file.neff"):
        return orig(_split_waits_json(bytes(bir_json)), tmpdir, neff_name=neff_name)

    bass_utils.compile_bir_kernel = patched
    bass2jax.compile_bir_kernel = patched
    bass_utils._waitsplit_installed = True


_install_waitsplit()

# ---------------------------------------------------------------------------
# Kernel builder
# ---------------------------------------------------------------------------
P = 128
C = 64
N_LAYERS = 3


def build_gcn(nn_nodes: int = 2048):
    """Single-core Bass program; runs SPMD on all 8 cores with per-core
    (per-batch) inputs."""
    NN = nn_nodes
    NB = NN // P            # node blocks (16)
    CH = 512                # aggregation i-chunk (one PSUM bank of f32)
    IC = NN // CH           # i-chunks (4)
    GS = CH // P            # slabs per i-chunk group (4)

    nc = bass.Bass()
    X_in = nc.declare_dram_parameter("X", [NN, C], f32, isOutput=False)
    A_in = nc.declare_dram_parameter("A", [NN, NN], f32, isOutput=False)
    W_in = {}
    b_in = {}
    for l in range(N_LAYERS):
        W_in[(l, "m")] = nc.declare_dram_parameter(f"Wm{l}", [C, C], f32, isOutput=False)
        W_in[(l, "s")] = nc.declare_dram_parameter(f"Ws{l}", [C, C], f32, isOutput=False)
        b_in[l] = nc.declare_dram_parameter(f"b{l}", [C], f32, isOutput=False)
    H_out = nc.declare_dram_parameter("H", [NN, C], f32, isOutput=True)

    with tile.TileContext(nc) as tc:
        with (
            tc.tile_pool(name="const", bufs=1) as const,
            tc.tile_pool(name="ht_pool", bufs=2) as ht_pool,
            tc.tile_pool(name="mn_pool", bufs=2) as mn_pool,
            tc.tile_pool(name="slab_pool", bufs=10) as slab_pool,
            tc.tile_pool(name="u_pool", bufs=2) as u_pool,
            tc.tile_pool(name="hb_pool", bufs=4) as hb_pool,
            tc.tile_pool(name="ps_tr", bufs=4, space="PSUM") as ps_tr,
            tc.tile_pool(name="ps_o", bufs=2, space="PSUM") as ps_o,
            tc.tile_pool(name="ps_m", bufs=2, space="PSUM") as ps_m,
        ):
            # ---- phase 0: constants, X, W loads ---------------------------
            ident = const.tile([P, P], f32, name="ident")
            id_i1 = nc.gpsimd.memset(ident, 0.0)
            id_i2 = nc.gpsimd.affine_select(
                out=ident, in_=ident,
                compare_op=mybir.AluOpType.not_equal,
                fill=1.0, base=0, pattern=[[-1, P]], channel_multiplier=1,
            )
            identb = const.tile([P, P], bf16, name="identb")
            idb = nc.vector.tensor_copy(identb, ident)

            # X in transpose-ready layout (256B-chunk descriptors -- slow
            # under the A flood, but nothing on the A queue waits for it;
            # only xT/mprod0 do, and they sit late enough in the PE stream)
            x_sb = const.tile([P, NB, C], f32, name="x_sb")
            x_dma = nc.sync.dma_start(
                x_sb, X_in[:].rearrange("(nb p) c -> p nb c", p=P))

            w_stage = {}
            w_dmas = []
            for l in range(N_LAYERS):
                for kind in ("m", "s"):
                    wst = const.tile([C, C], f32, name=f"wst_{l}{kind}")
                    w_dmas.append(nc.sync.dma_start(wst, W_in[(l, kind)][:]))
                    w_stage[(l, kind)] = wst
            b_sb = {}
            b_dmas = []
            for l in range(N_LAYERS):
                bt = const.tile([C, 1], f32, name=f"b_sb{l}")
                b_dmas.append(nc.sync.dma_start(
                    bt, b_in[l][:].rearrange("(p o) -> p o", o=1)))
                b_sb[l] = bt

            # ---- A slab loads: cast f32->bf16 during DMA (SWDGE) ----------
            a_sl = []
            a_dmas = []
            for s in range(NB):
                t = slab_pool.tile([P, NN], bf16, name=f"asl{s}", tag="aslab")
                d = nc.gpsimd.dma_start(t, A_in[s * P:(s + 1) * P, :])
                if s == 0:
                    # keep the gpsimd ident setup ahead of the descriptor
                    # generation flood for the A slabs
                    add_dep(d.ins, id_i2.ins, False, "ident before A gen")
                if s == 2:
                    # slabs 0-1 start immediately; slab 2+ waits until the
                    # small input DMAs have drained so x/w are not starved
                    # behind 16MB of A traffic (SWDGE is FIFO, so this
                    # pauses the whole A queue briefly)
                    for sd in (x_dma, *w_dmas):
                        add_dep(d.ins, sd.ins, True, "small DMAs before A")

                a_sl.append(t)
                a_dmas.append(d)

            # granular gates: xT needs x+ident only; w gates separate
            gate_x = nc.tensor.nop(nofuse=True)
            for d in (id_i1, id_i2, idb, x_dma):
                add_dep(gate_x.ins, d.ins, True, "x gate")
            gate_w = nc.tensor.nop(nofuse=True)
            for d in w_dmas:
                add_dep(gate_w.ins, d.ins, True, "w gate")

            # warm-up matmuls: engage the PE HAM clock-gate before real work
            warm_gate = nc.tensor.nop(nofuse=True)
            for d in (id_i1, id_i2):
                add_dep(warm_gate.ins, d.ins, True, "warmup gate")
            pwarm = ps_m.tile([P, C], f32, name="pwarm", tag="m")
            for wi in range(28):
                wmm = nc.tensor.matmul(pwarm[:C, :], ident[:, :C],
                                       ident[:, :C], start=True, stop=True,
                                       skip_group_check=True)
                if wi == 0:
                    add_dep(wmm.ins, warm_gate.ins, False, "after warmup gate")

            # ---- X^T -> Ht[0] (bf16), W^T (bf16) --------------------------
            # Ht[l]: transposed state, padded to 128 partitions with zeros
            # so the self-term matmul can run k=128 as the full-bank
            # accumulation opener (see emit_agg).
            Ht = [ht_pool.tile([P, NN], bf16, name=f"Ht{l}", tag="ht")
                  for l in range(N_LAYERS)]
            # bufs=2 ring: zeroing the bottom half of the first two tiles
            # covers both physical buffers; nothing ever writes it again.
            # On DVE: gpsimd must stay free for the A-slab descriptor gen.
            nc.vector.memset(Ht[0][C:, :], 0.0)
            nc.vector.memset(Ht[1][C:, :], 0.0)

            def emit_xT():
                for q in range(NB // 4):
                    pt = ps_tr.tile([P, 4 * P], f32, name="pt_x", tag="tr")
                    for si in range(4):
                        nb = q * 4 + si
                        t = nc.tensor.matmul(pt[:C, si * P:(si + 1) * P],
                                             x_sb[:, nb, :], ident,
                                             start=True, stop=True,
                                             skip_group_check=True)
                        if q == 0 and si == 0:
                            add_dep(t.ins, gate_x.ins, False, "after gate_x")
                    nc.vector.tensor_copy(
                        Ht[0][:C, q * 4 * P:(q + 1) * 4 * P],
                        pt[:C, :].rearrange("p (si i) -> p si i", si=4))

            wT = {}
            # self-term weights live in a [128, 128] zero-padded tile so the
            # self matmul runs k=128 x m=128 (writes the whole PSUM bank)
            wsp = {}
            for l in range(N_LAYERS):
                t = const.tile([P, C], bf16, name=f"wsp{l}")
                nc.vector.memset(t, 0.0)
                wsp[l] = t

            def emit_wT():
                first_w = True
                for (l, kind), wst in w_stage.items():
                    pw = ps_tr.tile([P, 4 * P], f32, name="pt_w", tag="tr")
                    t = nc.tensor.matmul(pw[:C, :C], wst, ident[:C, :C],
                                         start=True, stop=True,
                                         skip_group_check=True)
                    if first_w:
                        add_dep(t.ins, gate_w.ins, False, "after gate_w")
                        first_w = False
                    if kind == "s":
                        nc.vector.tensor_copy(wsp[l][:C, :C], pw[:C, :C])
                    else:
                        wt = const.tile([C, C], bf16, name=f"wT_{l}{kind}")
                        nc.vector.tensor_copy(wt, pw[:C, :C])
                        wT[(l, kind)] = wt

            # resident A^T [j-partition, j-block, i], bf16
            ATr = const.tile([P, NB, NN], bf16, name="ATr")

            mn_ctr = [0]

            def emit_mprod(l, mn, jbs=None, after=None):
                """M_l natural [N, C] blocks: lhsT = Ht[l] block, rhs = WmT.

                Emitted in groups of 4 into one PSUM bank with a single
                evacuation copy, so the copies never stall the matmul
                stream.  `after`: program-order pin so the scheduler cannot
                scatter these between a chunk's agg matmuls."""
                jbs = list(range(NB) if jbs is None else jbs)
                for q0 in range(0, len(jbs), 4):
                    grp = jbs[q0:q0 + 4]
                    pm = ps_m.tile([P, 4, C], f32, name="pm", tag="m")
                    for k, jb in enumerate(grp):
                        mm = nc.tensor.matmul(
                            pm[:, k, :], Ht[l][:C, jb * P:(jb + 1) * P],
                            wT[(l, "m")], start=True, stop=True,
                            skip_group_check=True)
                        if after is not None:
                            add_dep(mm.ins, after.ins, False, "pin after agg")
                            after = None
                    dst = mn[:, grp[0]:grp[0] + len(grp), :]
                    # alternate Mn copies DVE/ACT
                    if mn_ctr[0] % 2 == 0:
                        nc.scalar.copy(dst, pm[:, :len(grp), :])
                    else:
                        nc.vector.tensor_copy(dst, pm[:, :len(grp), :])
                    mn_ctr[0] += 1

            # bridge dummies: keep PE warm across the wait for slab 0
            pbr = ps_m.tile([P, C], f32, name="pbr", tag="m")
            for _bi in range(6):
                nc.tensor.matmul(pbr[:C, :], ident[:, :C], ident[:, :C],
                                 start=True, stop=True, skip_group_check=True)

            # ---- A transposes: plain-matmul vs bf16 identity --------------
            evac_ctr = [0]

            def emit_slab_transposes(s):
                """16 block transposes of slab s into ATr[:, :, s*128:...]."""
                for q in range(NB // 4):
                    pt = ps_tr.tile([P, 4 * P], f32, name="pt_a", tag="tr")
                    for si in range(4):
                        jb = q * 4 + si
                        nc.tensor.matmul(
                            pt[:, si * P:(si + 1) * P],
                            a_sl[s][:, jb * P:(jb + 1) * P],
                            identb, start=True, stop=True,
                            skip_group_check=True)
                    dst = ATr[:, q * 4:(q + 1) * 4, s * P:(s + 1) * P]
                    src = pt[:, :].rearrange("p (si i) -> p si i", si=4)
                    # 2:1 DVE:ACT split on the cast evacuations
                    if evac_ctr[0] % 3 != 2:
                        nc.vector.tensor_copy(dst, src)
                    else:
                        nc.scalar.copy(dst, src)
                    evac_ctr[0] += 1

            ho_tiles = {}

            def emit_evac(l, g, po):
                # col-packed halves: out = po[0:64] + po[64:128] + b.
                # Engines read at most one non-scalar PSUM input per op.
                cs = slice(g * CH, (g + 1) * CH)
                v = u_pool.tile([C, CH], f32, name="v", tag="v")
                nc.scalar.activation(v, po[C:2 * C, :],
                                     mybir.ActivationFunctionType.Identity,
                                     bias=b_sb[l], scale=1.0)
                if l < N_LAYERS - 1:
                    u = u_pool.tile([C, CH], f32, name="u", tag="u")
                    nc.vector.tensor_tensor(u, po[:C, :], v,
                                            mybir.AluOpType.add)
                    nc.vector.tensor_scalar(
                        Ht[l + 1][:C, cs], u, 0.0, None, mybir.AluOpType.max)
                    return
                # final layer: bias only (no relu); transpose-out deferred
                # one chunk so the PE is not stalled on this evac chain
                ho = u_pool.tile([C, CH], f32, name="ho", tag="ho")
                nc.vector.tensor_tensor(ho, po[:C, :], v,
                                        mybir.AluOpType.add)
                ho_tiles[g] = ho

            def emit_final_out(g, after=None):
                ho = ho_tiles.pop(g)
                hb = hb_pool.tile([P, CH // P, C], f32, name="hb", tag="hb")
                for k in range(CH // P):
                    ph = ps_tr.tile([P, 4 * P], f32, name="ph", tag="tr")
                    mm = nc.tensor.matmul(ph[:, :C], ho[:, k * P:(k + 1) * P],
                                          ident[:C, :C], start=True, stop=True,
                                          skip_group_check=True)
                    if after is not None:
                        add_dep(mm.ins, after.ins, False, "pin after agg")
                        after = None
                    nc.vector.tensor_copy(hb[:, k, :], ph[:, :C])
                r0 = g * CH
                nc.sync.dma_start(
                    H_out[r0:r0 + CH, :].rearrange("(k p) c -> p k c", p=P),
                    hb)

            def emit_agg(l, g, mn):
                # col-packed: even j-blocks -> psum partitions 0:64, odd ->
                # 64:128; concurrent in the array's column groups, which
                # also lets LDWEIGHTS overlap the other half's matmul.
                cs = slice(g * CH, (g + 1) * CH)
                po = ps_o.tile([P, CH], f32, name="po", tag="o")
                # The k=128 m=64 self-term (zero-padded weights rows 64:128)
                # opens the even col-group chain; jb1 opens the odd chain
                # concurrently in the other col group.  j-blocks then
                # alternate col groups so each LDWEIGHTS hides under the
                # other group's streaming.
                last = nc.tensor.matmul(po[:C, :], wsp[l], Ht[l][:, cs],
                                        start=True, stop=False,
                                        skip_group_check=True)
                seq = []
                for k in range(NB // 2):
                    seq.append((2 * k + 1, k == 0, 2 * k + 1 == NB - 1))
                    seq.append((2 * k, False, 2 * k == NB - 2))
                for jb, st, sp in seq:
                    h = jb % 2
                    last = nc.tensor.matmul(
                        po[h * C:(h + 1) * C, :],
                        mn[:, jb, :], ATr[:, jb, cs],
                        start=st, stop=sp, skip_group_check=True)
                emit_evac(l, g, po)
                return last

            # ---- layer 1, pipelined with the A load/transpose -------------
            mns = [mn_pool.tile([P, NB, C], bf16, name=f"mn{l}", tag="mn")
                   for l in range(N_LAYERS)]

            # Mprods for layer l+1 chunk g are emitted one chunk LATE (after
            # agg of chunk g+1) so the PE never stalls on chunk g's
            # ACT/DVE evacuation chain.  wT/mprod0 are emitted after slab
            # 0's transposes: slab 0 usually lands before the W inputs.
            GB = NB // IC
            for g in range(IC):
                for s in range(g * GS, (g + 1) * GS):
                    emit_slab_transposes(s)
                    if s == 0:
                        emit_xT()
                    if s == 1:
                        emit_wT()
                        emit_mprod(0, mns[0])
                    if g > 0 and s % GS == 0:
                        la = emit_agg(0, g - 1, mns[0])
                        if g > 1:
                            jb0 = (g - 2) * GB
                            emit_mprod(1, mns[1], range(jb0, jb0 + GB),
                                       after=la)
            la = emit_agg(0, IC - 1, mns[0])
            emit_mprod(1, mns[1], range((IC - 2) * GB, (IC - 1) * GB),
                       after=la)
            emit_mprod(1, mns[1], range((IC - 1) * GB, NB))

            # ---- layer 2 --------------------------------------------------
            for g in range(IC):
                la = emit_agg(1, g, mns[1])
                if g > 0:
                    jb0 = (g - 1) * GB
                    emit_mprod(2, mns[2], range(jb0, jb0 + GB), after=la)
            emit_mprod(2, mns[2], range((IC - 1) * GB, NB))

            # ---- layer 3 + transposed output ------------------------------
            for g in range(IC):
                la = emit_agg(2, g, mns[2])
                if g > 0:
                    emit_final_out(g - 1, after=la)
            emit_final_out(IC - 1)

    return nc


# ---------------------------------------------------------------------------
# Harness entry point
# ---------------------------------------------------------------------------
_NC_CACHE = {}


def _get_nc(nn_nodes):
    if nn_nodes not in _NC_CACHE:
        _NC_CACHE[nn_nodes] = build_gcn(nn_nodes)
    return _NC_CACHE[nn_nodes]


def kernel(X, A, Wm0, Ws0, b0, Wm1, Ws1, b1, Wm2, Ws2, b2, _trace=False):
    from concourse.bass_utils import run_bass_kernel_spmd

    X = np.ascontiguousarray(np.asarray(X, dtype=np.float32))
    A = np.ascontiguousarray(np.asarray(A, dtype=np.float32))
    B, NN, _C = X.shape
    assert B == 8, f"expected batch 8 (one per core), got {B}"

    shared = {
        "Wm0": np.ascontiguousarray(np.asarray(Wm0, np.float32)),
        "Ws0": np.ascontiguousarray(np.asarray(Ws0, np.float32)),
        "b0": np.ascontiguousarray(np.asarray(b0, np.float32)),
        "Wm1": np.ascontiguousarray(np.asarray(Wm1, np.float32)),
        "Ws1": np.ascontiguousarray(np.asarray(Ws1, np.float32)),
        "b1": np.ascontiguousarray(np.asarray(b1, np.float32)),
        "Wm2": np.ascontiguousarray(np.asarray(Wm2, np.float32)),
        "Ws2": np.ascontiguousarray(np.asarray(Ws2, np.float32)),
        "b2": np.ascontiguousarray(np.asarray(b2, np.float32)),
    }
    nc = _get_nc(NN)
    in_maps = [dict(shared, X=X[b], A=A[b]) for b in range(B)]
    res = run_bass_kernel_spmd(nc, in_maps, core_ids=list(range(B)),
                               trace=_trace)
    out = np.stack([res.results[b]["H"] for b in range(B)], axis=0)
    if _trace:
        return out, res
    return out
